# revision 11
# baseline (speedup 1.0000x reference)
"""Trainium2 Bass kernel for a quantized Mistral-style SwiGLU MLP.

Reference computation (per token x of dim HIDDEN=4096):
    g = x @ (gate_wq * gate_scale[:, None]).T      # [INTER]
    u = x @ (up_wq   * up_scale[:, None]).T        # [INTER]
    h = silu(g) * u
    y = h @ (down_wq * down_scale[:, None]).T      # [HIDDEN]

Sharding across 8 NeuronCores: DP4 (token groups of 2048) x TP2 (intermediate
shards of 7168).  Each core runs the same SPMD program:
  phase 1: xT resident in SBUF as KH per-k chunk tiles (feature-major
           [hid, tok]); gate/up weight tiles stream on the ACT HWDGE queue
           while x streams on the SP queue; PE matmuls accumulate g,u in
           PSUM (k-outer / q-inner: stationary held across the MQ moving
           passes, consecutive matmuls on different PSUM banks); ACT applies
           silu(gate_scale*g); DVE forms h = (up_scale*u) * silu(...) in
           bf16; h goes to a DRAM bounce buffer.  o=0 runs gate/up
           interleaved per k so the PE chases the x chunk DMAs instead of
           waiting for the full x load.
  phase 2: h streamed back per k-quarter as per-kk chunk tiles (sync queue;
           next quarter prefetches during the current one); down weight
           tiles stream on the ACT queue; PE matmuls accumulate each
           quarter's [hid, tok] fp32 partial in PSUM, and the 4 partials are
           summed in DRAM via SWDGE accumulate-DMA (output buffers are
           zero-initialized by the SPMD runner).  The first two output
           tiles of quarter 0 run jointly kk-outer to chase the h chunk
           loads.
Host sums the TP pair, applies down_scale, and re-assembles [B, S, HIDDEN].

All weights are fed to the device as exact bf16 integers (values in
[-128,127] are exactly representable); scales stay fp32 and are applied
per-partition on chip (gate/up) or on host (down).
"""

import numpy as np
import ml_dtypes

import concourse.bacc as bacc
import concourse.mybir as mybir
import concourse.tile as tile
from concourse.bass_utils import run_bass_kernel_spmd

BF16 = ml_dtypes.bfloat16
BF = mybir.dt.bfloat16
F32 = mybir.dt.float32

N_CORES = 8
DP, TP = 4, 2
HIDDEN, INTER = 4096, 14336
B, S = 4, 2048

P = 128
FD = 512   # matmul moving inner dim (one PSUM bank of fp32; ISA cap)
FDP = 520  # padded segment pitch: keeps the moving AP 3D ([MQ, 512] with
           # pitch 520) so bass's .opt() can't merge it into an illegal
           # [2048] flat dim; one matmul then streams all MQ*FD columns


def build_module(hidden, inter_sh, m, kq_splits=None):
    """Build the per-core SPMD Bass module.

    hidden:   full hidden dim (contraction of phase 1, output of phase 2)
    inter_sh: this core's intermediate-dim shard
    m:        tokens per core
    """
    KH = hidden // P        # phase-1 contraction chunks
    NO = inter_sh // P      # phase-1 output tiles (inter)
    OH = hidden // P        # phase-2 output tiles (hid)
    MQ = m // FD            # moving passes per psum row
    KQ = 4 if NO % 4 == 0 else 1   # phase-2 k-quarters (h resident per quarter)
    KK = NO // KQ
    assert m % FD == 0

    nc = bacc.Bacc("TRN2", target_bir_lowering=False, debug=False,
                   num_devices=N_CORES)

    xT_d = nc.dram_tensor("xT", [P, KH, m // FD, FD], BF,
                      kind="ExternalInput").ap()
    gw_d = nc.dram_tensor("gw", [NO, P, KH * P], BF, kind="ExternalInput").ap()
    uw_d = nc.dram_tensor("uw", [NO, P, KH * P], BF, kind="ExternalInput").ap()
    dw_d = nc.dram_tensor("dw", [OH, P, NO * P], BF, kind="ExternalInput").ap()
    gs_d = nc.dram_tensor("gs", [P, NO], F32, kind="ExternalInput").ap()
    us_d = nc.dram_tensor("us", [P, NO], F32, kind="ExternalInput").ap()
    y_d = nc.dram_tensor("y", [OH, P, m], F32, kind="ExternalOutput").ap()

    mult = mybir.AluOpType.mult
    silu = mybir.ActivationFunctionType.Silu

    with tile.TileContext(nc) as tc:
        with tc.tile_pool(name="const", bufs=1) as cpool, \
             tc.tile_pool(name="dram", bufs=1, space="DRAM") as dpool:
            gs_sb = cpool.tile([P, NO], F32, tag="gs")
            us_sb = cpool.tile([P, NO], F32, tag="us")
            nc.sync.dma_start(out=gs_sb[:], in_=gs_d[:])
            nc.sync.dma_start(out=us_sb[:], in_=us_d[:])
            h_d = dpool.tile([NO, P, m], BF)
            h_dv = h_d.rearrange("o p (q f) -> o p q f", f=FD)

            # ---------------- phase 1: h = silu(gs*g) * (us*u) ----------
            with tc.tile_pool(name="xp", bufs=1) as xp, \
                 tc.tile_pool(name="wp", bufs=2) as wp, \
                 tc.tile_pool(name="sp", bufs=2) as sp, \
                 tc.tile_pool(name="pp", bufs=1, space="PSUM") as pp:
                xk = []
                for k in range(KH):
                    t = xp.tile([P, MQ, FDP], BF, tag=f"x{k}")
                    nc.sync.dma_start(out=t[:, :, 0:FD], in_=xT_d[:, k])
                    xk.append(t)
                for o in range(NO):
                    gwt = wp.tile([P, KH * P], BF, tag="gw")
                    uwt = wp.tile([P, KH * P], BF, tag="uw")
                    nc.scalar.dma_start(out=gwt[:], in_=gw_d[o])
                    nc.scalar.dma_start(out=uwt[:], in_=uw_d[o])
                    pg = pp.tile([P, m], F32, tag="pg")
                    pu = pp.tile([P, m], F32, tag="pu")
                    if o == 0:
                        # chase the x chunk DMAs: both matrices per k chunk
                        for k in range(KH):
                            nc.tensor.matmul(
                                pg[:], gwt[:, k * P:(k + 1) * P],
                                xk[k][:, :, 0:FD],
                                start=(k == 0), stop=(k == KH - 1))
                            nc.tensor.matmul(
                                pu[:], uwt[:, k * P:(k + 1) * P],
                                xk[k][:, :, 0:FD],
                                start=(k == 0), stop=(k == KH - 1))
                    else:
                        for k in range(KH):
                            nc.tensor.matmul(
                                pg[:], gwt[:, k * P:(k + 1) * P],
                                xk[k][:, :, 0:FD],
                                start=(k == 0), stop=(k == KH - 1))
                        for k in range(KH):
                            nc.tensor.matmul(
                                pu[:], uwt[:, k * P:(k + 1) * P],
                                xk[k][:, :, 0:FD],
                                start=(k == 0), stop=(k == KH - 1))
                    sg = sp.tile([P, m], BF, tag="sg")
                    nc.scalar.activation(sg[:], pg[:], silu,
                                         scale=gs_sb[:, o:o + 1])
                    hb = sp.tile([P, m], BF, tag="hb")
                    nc.vector.scalar_tensor_tensor(
                        hb[:], pu[:], us_sb[:, o:o + 1], sg[:], mult, mult)
                    nc.sync.dma_start(out=h_d[o], in_=hb[:])

            # ---- phase 2: y += h[kq] @ down[kq], DMA-accumulated over kq ----
            with tc.tile_pool(name="hqp", bufs=2) as hqp, \
                 tc.tile_pool(name="dwp", bufs=3) as dwp, \
                 tc.tile_pool(name="yop", bufs=2) as yop, \
                 tc.tile_pool(name="pp2", bufs=2, space="PSUM") as pp2:
                for kq in range(KQ):
                    hk = []
                    for kk in range(KK):
                        t = hqp.tile([P, MQ, FDP], BF, tag=f"h{kk}")
                        nc.sync.dma_start(out=t[:, :, 0:FD],
                                          in_=h_dv[kq * KK + kk])
                        hk.append(t)

                    def dw_load(o):
                        dwt = dwp.tile([P, KK * P], BF, tag="dw")
                        nc.scalar.dma_start(
                            out=dwt[:],
                            in_=dw_d[o][:, kq * KK * P:(kq + 1) * KK * P])
                        return dwt

                    def py_out(o, py):
                        yo = yop.tile([P, m], F32, tag="yo")
                        nc.vector.tensor_copy(yo[:], py[:])
                        if KQ == 1:
                            nc.sync.dma_start(out=y_d[o], in_=yo[:])
                        else:
                            # accumulate partials straight into DRAM (SWDGE);
                            # output buffers are zero-init'd by the runner
                            nc.gpsimd.dma_start(out=y_d[o], in_=yo[:],
                                                accum_op=mybir.AluOpType.add)

                    o0 = 0
                    if kq == 0 and OH >= 2 and KQ > 1:
                        # chase the h chunk DMAs: first two output tiles
                        # jointly, kk-outer (8 matmuls per arriving chunk)
                        dwt0, dwt1 = dw_load(0), dw_load(1)
                        py0 = pp2.tile([P, m], F32, tag="py")
                        py1 = pp2.tile([P, m], F32, tag="py")
                        for kk in range(KK):
                            nc.tensor.matmul(
                                py0[:], dwt0[:, kk * P:(kk + 1) * P],
                                hk[kk][:, :, 0:FD],
                                start=(kk == 0), stop=(kk == KK - 1))
                            nc.tensor.matmul(
                                py1[:], dwt1[:, kk * P:(kk + 1) * P],
                                hk[kk][:, :, 0:FD],
                                start=(kk == 0), stop=(kk == KK - 1))
                        py_out(0, py0)
                        py_out(1, py1)
                        o0 = 2
                    for o in range(o0, OH):
                        dwt = dw_load(o)
                        py = pp2.tile([P, m], F32, tag="py")
                        for kk in range(KK):
                            nc.tensor.matmul(
                                py[:], dwt[:, kk * P:(kk + 1) * P],
                                hk[kk][:, :, 0:FD],
                                start=(kk == 0), stop=(kk == KK - 1))
                        py_out(o, py)

    nc.compile()
    return nc


def prep_core_inputs(x_flat, gate_wq, gate_scale, up_wq, up_scale, down_wq,
                     hidden, inter, dp, tp, kq_splits=None):
    """Shard + repack full inputs into per-core input maps (list of dicts)."""
    n_tok = x_flat.shape[0]
    m = n_tok // dp
    inter_sh = inter // tp
    KH = hidden // P
    NO = inter_sh // P
    OH = hidden // P

    # per-TP-shard weight packs (shared by all DP groups)
    packs = []
    for s in range(tp):
        lo, hi = s * inter_sh, (s + 1) * inter_sh
        gq = gate_wq[lo:hi].astype(BF16)
        uq = up_wq[lo:hi].astype(BF16)
        # [o,c,k,p] -> [o,p,k,c] -> [NO, P, KH*P]
        gw = np.ascontiguousarray(
            gq.reshape(NO, P, KH, P).transpose(0, 3, 2, 1)).reshape(NO, P, KH * P)
        uw = np.ascontiguousarray(
            uq.reshape(NO, P, KH, P).transpose(0, 3, 2, 1)).reshape(NO, P, KH * P)
        dq = down_wq[:, lo:hi].astype(BF16)
        # [o,c,j,p] -> [o,p,j,c] -> [OH, P, NO*P]
        dw = np.ascontiguousarray(
            dq.reshape(OH, P, NO, P).transpose(0, 3, 2, 1)).reshape(OH, P, NO * P)
        gs = np.ascontiguousarray(gate_scale[lo:hi].reshape(NO, P).T)
        us = np.ascontiguousarray(up_scale[lo:hi].reshape(NO, P).T)
        packs.append(dict(gw=gw, uw=uw, dw=dw, gs=gs, us=us))

    in_maps = []
    for g in range(dp):
        xg = x_flat[g * m:(g + 1) * m]  # [m, hidden]
        xT = np.ascontiguousarray(xg.T.astype(BF16)).reshape(P * KH, m)
        # [hidden, m] with hidden = k*P + p -> [P, KH, m]
        xT = np.ascontiguousarray(
            xT.reshape(KH, P, m).transpose(1, 0, 2))
        for s in range(tp):
            in_maps.append({"xT": xT, **packs[s]})
    return in_maps


_NC_CACHE = {}


def _get_module():
    key = "full"
    if key not in _NC_CACHE:
        _NC_CACHE[key] = build_module(HIDDEN, INTER // TP, (B * S) // DP)
    return _NC_CACHE[key]


def kernel(x, gate_wq, gate_scale, up_wq, up_scale, down_wq, down_scale,
           _return_results=False):
    x = np.asarray(x)
    x_flat = x.reshape(B * S, HIDDEN)
    in_maps = prep_core_inputs(
        x_flat, np.asarray(gate_wq), np.asarray(gate_scale),
        np.asarray(up_wq), np.asarray(up_scale), np.asarray(down_wq),
        HIDDEN, INTER, DP, TP)

    nc = _get_module()
    res = run_bass_kernel_spmd(nc, in_maps, list(range(N_CORES)))

    m = (B * S) // DP
    y = np.empty((B * S, HIDDEN), np.float32)
    ds = np.asarray(down_scale).astype(np.float32)
    for g in range(DP):
        acc = None
        for s in range(TP):
            part = res.results[g * TP + s]["y"]  # [OH, P, m]
            acc = part if acc is None else acc + part
        # [OH, P, m] -> [hidden, m] -> [m, hidden]
        y[g * m:(g + 1) * m] = acc.reshape(HIDDEN, m).T
    y *= ds[None, :]
    out = y.reshape(B, S, HIDDEN)
    if _return_results:
        return out, res
    return out


# revision 21
# speedup vs baseline: 1.1918x; 1.1918x over previous
"""Trainium2 Bass kernel for a quantized Mistral-style SwiGLU MLP.

Reference computation (per token x of dim HIDDEN=4096):
    g = x @ (gate_wq * gate_scale[:, None]).T      # [INTER]
    u = x @ (up_wq   * up_scale[:, None]).T        # [INTER]
    h = silu(g) * u
    y = h @ (down_wq * down_scale[:, None]).T      # [HIDDEN]

Sharding across 8 NeuronCores: DP4 (token groups of 2048) x TP2 (intermediate
shards of 7168).  Each core runs the same SPMD program:
  phase 1: xT resident in SBUF as KH per-k chunk tiles (feature-major
           [hid, tok]); gate/up weight tiles stream on the ACT HWDGE queue
           while x streams on the SP queue; PE matmuls accumulate g,u in
           PSUM (k-outer / q-inner: stationary held across the MQ moving
           passes, consecutive matmuls on different PSUM banks); ACT applies
           silu(gate_scale*g); DVE forms h = (up_scale*u) * silu(...) in
           bf16; h goes to a DRAM bounce buffer.  o=0 runs gate/up
           interleaved per k so the PE chases the x chunk DMAs instead of
           waiting for the full x load.
  phase 2: h streamed back per k-quarter as per-kk chunk tiles (sync queue;
           next quarter prefetches during the current one); down weight
           tiles stream on the ACT queue; PE matmuls accumulate each
           quarter's [hid, tok] fp32 partial in PSUM, and the 4 partials are
           summed in DRAM via SWDGE accumulate-DMA (output buffers are
           zero-initialized by the SPMD runner).  The first two output
           tiles of quarter 0 run jointly kk-outer to chase the h chunk
           loads.
Host sums the TP pair, applies down_scale, and re-assembles [B, S, HIDDEN].

All weights are fed to the device as exact bf16 integers (values in
[-128,127] are exactly representable); scales stay fp32 and are applied
per-partition on chip (gate/up) or on host (down).
"""

import numpy as np
import ml_dtypes

import concourse.bacc as bacc
import concourse.mybir as mybir
import concourse.tile as tile
from concourse.bass_utils import run_bass_kernel_spmd

BF16 = ml_dtypes.bfloat16
BF = mybir.dt.bfloat16
F32 = mybir.dt.float32

N_CORES = 8
DP, TP = 4, 2
HIDDEN, INTER = 4096, 14336
B, S = 4, 2048

P = 128
FD = 512  # matmul moving free dim (ISA caps total moving columns at 512)

# walrus ships with the redundant-LDWEIGHTS optimization off; our k-outer /
# q-inner matmul streams repeat each stationary operand MQ times in a row,
# which is exactly the pattern that pass optimizes.  The pass refuses
# modules with standalone Ldweights instructions, but the bass serializer
# unconditionally splits every self-loading Matmult into an Ldweights +
# Matmult pair (each BIR instruction may carry at most one semaphore wait).
# So before invoking walrus we re-merge each pair back into a self-loading
# Matmult, moving excess waits onto a PE EventSemaphore (which may carry
# two), and flip --enable-ldw-opt on.  Any assumption violation falls back
# to the untouched BIR with the flag off.
import json as _json
import os as _os
import concourse.bass_utils as _bass_utils


def _merge_ldweights(bir):
    n_es = 0
    for fn in bir["functions"]:
        for blk in fn["blocks"]:
            insts = blk["instructions"]
            out = []
            pending = None
            for inst in insts:
                if inst["opcode"] == "Ldweights":
                    if pending is not None:
                        return None
                    pending = inst
                    continue
                if pending is not None:
                    if inst["opcode"] != "Matmult":
                        return None
                    if inst.get("ldweights"):
                        return None
                    w_mm = inst["ins"][1]
                    w_ld = pending["ins"][0]
                    if (w_mm["memref"] != w_ld["memref"]
                            or w_mm["offset"] != w_ld["offset"]
                            or w_mm["ap"] != w_ld["ap"]):
                        return None
                    if pending["sync_info"]["on_update"]:
                        return None
                    waits = (pending["sync_info"]["on_wait"]
                             + inst["sync_info"]["on_wait"])
                    inst["ldweights"] = True
                    if len(waits) <= 1:
                        inst["sync_info"]["on_wait"] = waits
                    elif len(waits) <= 2:
                        n_es += 1
                        out.append({
                            "debug": inst.get("debug", 0),
                            "engine": "PE",
                            "ins": [], "outs": [],
                            "name": f"ldwmerge_es_{n_es}",
                            "opcode": "EventSemaphore",
                            "sync_info": {"on_update": [], "on_wait": waits},
                        })
                        inst["sync_info"]["on_wait"] = []
                    else:
                        return None
                    pending = None
                out.append(inst)
            if pending is not None:
                return None
            blk["instructions"] = out
    return bir


if not getattr(_bass_utils, "_ldw_opt_patched", False):
    _orig_run_command = _bass_utils.run_command

    def _run_command_ldw(cmd, *args, **kwargs):
        if "--enable-ldw-opt=false" in cmd:
            cwd = kwargs.get("cwd")
            bir_path = _os.path.join(cwd, "bir.json") if cwd else None
            merged = None
            if bir_path and _os.path.exists(bir_path):
                try:
                    with open(bir_path) as f:
                        merged = _merge_ldweights(_json.load(f))
                except Exception:
                    merged = None
            if merged is not None:
                with open(bir_path, "w") as f:
                    _json.dump(merged, f)
                cmd = ["--enable-ldw-opt=true"
                       if c == "--enable-ldw-opt=false" else c for c in cmd]
        return _orig_run_command(cmd, *args, **kwargs)

    _bass_utils.run_command = _run_command_ldw
    _bass_utils._ldw_opt_patched = True


def build_module(hidden, inter_sh, m, kq_splits=None):
    """Build the per-core SPMD Bass module.

    hidden:   full hidden dim (contraction of phase 1, output of phase 2)
    inter_sh: this core's intermediate-dim shard
    m:        tokens per core
    """
    KH = hidden // P        # phase-1 contraction chunks
    NO = inter_sh // P      # phase-1 output tiles (inter)
    OH = hidden // P        # phase-2 output tiles (hid)
    MQ = m // FD            # moving passes per psum row
    KQ = 4 if NO % 4 == 0 else 1   # phase-2 k-quarters (h resident per quarter)
    KK = NO // KQ
    assert m % FD == 0

    nc = bacc.Bacc("TRN2", target_bir_lowering=False, debug=False,
                   num_devices=N_CORES)

    xT_d = nc.dram_tensor("xT", [P, KH, m], BF, kind="ExternalInput").ap()
    gw_d = nc.dram_tensor("gw", [NO, P, KH * P], BF, kind="ExternalInput").ap()
    uw_d = nc.dram_tensor("uw", [NO, P, KH * P], BF, kind="ExternalInput").ap()
    dw_d = nc.dram_tensor("dw", [OH, P, NO * P], BF, kind="ExternalInput").ap()
    gs_d = nc.dram_tensor("gs", [P, NO], F32, kind="ExternalInput").ap()
    us_d = nc.dram_tensor("us", [P, NO], F32, kind="ExternalInput").ap()
    y_d = nc.dram_tensor("y", [OH, P, m], F32, kind="ExternalOutput").ap()

    mult = mybir.AluOpType.mult
    silu = mybir.ActivationFunctionType.Silu

    with tile.TileContext(nc) as tc:
        with tc.tile_pool(name="const", bufs=1) as cpool, \
             tc.tile_pool(name="dram", bufs=1, space="DRAM") as dpool:
            gs_sb = cpool.tile([P, NO], F32, tag="gs")
            us_sb = cpool.tile([P, NO], F32, tag="us")
            nc.sync.dma_start(out=gs_sb[:], in_=gs_d[:])
            nc.sync.dma_start(out=us_sb[:], in_=us_d[:])
            h_d = dpool.tile([NO, P, m], BF)

            # ---------------- phase 1: h = silu(gs*g) * (us*u) ----------
            with tc.tile_pool(name="xp", bufs=1) as xp, \
                 tc.tile_pool(name="wp", bufs=2) as wp, \
                 tc.tile_pool(name="sp", bufs=2) as sp, \
                 tc.tile_pool(name="pp", bufs=1, space="PSUM") as pp:
                xk = []
                for k in range(KH):
                    t = xp.tile([P, m], BF, tag=f"x{k}")
                    nc.sync.dma_start(out=t[:], in_=xT_d[:, k])
                    xk.append(t)
                for o in range(NO):
                    gwt = wp.tile([P, KH * P], BF, tag="gw")
                    uwt = wp.tile([P, KH * P], BF, tag="uw")
                    nc.scalar.dma_start(out=gwt[:], in_=gw_d[o])
                    nc.scalar.dma_start(out=uwt[:], in_=uw_d[o])
                    pg = pp.tile([P, m], F32, tag="pg")
                    pu = pp.tile([P, m], F32, tag="pu")
                    if o == 0:
                        # chase the x chunk DMAs: both matrices per k chunk
                        for k in range(KH):
                            for q in range(MQ):
                                nc.tensor.matmul(
                                    pg[:, q * FD:(q + 1) * FD],
                                    gwt[:, k * P:(k + 1) * P],
                                    xk[k][:, q * FD:(q + 1) * FD],
                                    start=(k == 0), stop=(k == KH - 1))
                            for q in range(MQ):
                                nc.tensor.matmul(
                                    pu[:, q * FD:(q + 1) * FD],
                                    uwt[:, k * P:(k + 1) * P],
                                    xk[k][:, q * FD:(q + 1) * FD],
                                    start=(k == 0), stop=(k == KH - 1))
                    else:
                        for k in range(KH):
                            for q in range(MQ):
                                nc.tensor.matmul(
                                    pg[:, q * FD:(q + 1) * FD],
                                    gwt[:, k * P:(k + 1) * P],
                                    xk[k][:, q * FD:(q + 1) * FD],
                                    start=(k == 0), stop=(k == KH - 1))
                        for k in range(KH):
                            for q in range(MQ):
                                nc.tensor.matmul(
                                    pu[:, q * FD:(q + 1) * FD],
                                    uwt[:, k * P:(k + 1) * P],
                                    xk[k][:, q * FD:(q + 1) * FD],
                                    start=(k == 0), stop=(k == KH - 1))
                    sg = sp.tile([P, m], BF, tag="sg")
                    nc.scalar.activation(sg[:], pg[:], silu,
                                         scale=gs_sb[:, o:o + 1])
                    hb = sp.tile([P, m], BF, tag="hb")
                    nc.vector.scalar_tensor_tensor(
                        hb[:], pu[:], us_sb[:, o:o + 1], sg[:], mult, mult)
                    nc.sync.dma_start(out=h_d[o], in_=hb[:])

            # ---- phase 2: y += h[kq] @ down[kq], DMA-accumulated over kq ----
            with tc.tile_pool(name="hqp", bufs=2) as hqp, \
                 tc.tile_pool(name="dwp", bufs=3) as dwp, \
                 tc.tile_pool(name="yop", bufs=2) as yop, \
                 tc.tile_pool(name="pp2", bufs=2, space="PSUM") as pp2:
                for kq in range(KQ):
                    hk = []
                    for kk in range(KK):
                        t = hqp.tile([P, m], BF, tag=f"h{kk}")
                        nc.sync.dma_start(out=t[:], in_=h_d[kq * KK + kk])
                        hk.append(t)

                    def dw_load(o):
                        dwt = dwp.tile([P, KK * P], BF, tag="dw")
                        nc.scalar.dma_start(
                            out=dwt[:],
                            in_=dw_d[o][:, kq * KK * P:(kq + 1) * KK * P])
                        return dwt

                    def py_out(o, py):
                        yo = yop.tile([P, m], F32, tag="yo")
                        nc.vector.tensor_copy(yo[:], py[:])
                        if KQ == 1:
                            nc.sync.dma_start(out=y_d[o], in_=yo[:])
                        else:
                            # accumulate partials straight into DRAM (SWDGE);
                            # output buffers are zero-init'd by the runner
                            nc.gpsimd.dma_start(out=y_d[o], in_=yo[:],
                                                accum_op=mybir.AluOpType.add)

                    o0 = 0
                    if kq == 0 and OH >= 2 and KQ > 1:
                        # chase the h chunk DMAs: first two output tiles
                        # jointly, kk-outer (8 matmuls per arriving chunk)
                        dwt0, dwt1 = dw_load(0), dw_load(1)
                        py0 = pp2.tile([P, m], F32, tag="py")
                        py1 = pp2.tile([P, m], F32, tag="py")
                        for kk in range(KK):
                            for q in range(MQ):
                                nc.tensor.matmul(
                                    py0[:, q * FD:(q + 1) * FD],
                                    dwt0[:, kk * P:(kk + 1) * P],
                                    hk[kk][:, q * FD:(q + 1) * FD],
                                    start=(kk == 0), stop=(kk == KK - 1))
                            for q in range(MQ):
                                nc.tensor.matmul(
                                    py1[:, q * FD:(q + 1) * FD],
                                    dwt1[:, kk * P:(kk + 1) * P],
                                    hk[kk][:, q * FD:(q + 1) * FD],
                                    start=(kk == 0), stop=(kk == KK - 1))
                        py_out(0, py0)
                        py_out(1, py1)
                        o0 = 2
                    for o in range(o0, OH):
                        dwt = dw_load(o)
                        py = pp2.tile([P, m], F32, tag="py")
                        for kk in range(KK):
                            for q in range(MQ):
                                nc.tensor.matmul(
                                    py[:, q * FD:(q + 1) * FD],
                                    dwt[:, kk * P:(kk + 1) * P],
                                    hk[kk][:, q * FD:(q + 1) * FD],
                                    start=(kk == 0), stop=(kk == KK - 1))
                        py_out(o, py)

    nc.compile()
    return nc


def prep_core_inputs(x_flat, gate_wq, gate_scale, up_wq, up_scale, down_wq,
                     hidden, inter, dp, tp, kq_splits=None):
    """Shard + repack full inputs into per-core input maps (list of dicts)."""
    n_tok = x_flat.shape[0]
    m = n_tok // dp
    inter_sh = inter // tp
    KH = hidden // P
    NO = inter_sh // P
    OH = hidden // P

    # per-TP-shard weight packs (shared by all DP groups)
    packs = []
    for s in range(tp):
        lo, hi = s * inter_sh, (s + 1) * inter_sh
        gq = gate_wq[lo:hi].astype(BF16)
        uq = up_wq[lo:hi].astype(BF16)
        # [o,c,k,p] -> [o,p,k,c] -> [NO, P, KH*P]
        gw = np.ascontiguousarray(
            gq.reshape(NO, P, KH, P).transpose(0, 3, 2, 1)).reshape(NO, P, KH * P)
        uw = np.ascontiguousarray(
            uq.reshape(NO, P, KH, P).transpose(0, 3, 2, 1)).reshape(NO, P, KH * P)
        dq = down_wq[:, lo:hi].astype(BF16)
        # [o,c,j,p] -> [o,p,j,c] -> [OH, P, NO*P]
        dw = np.ascontiguousarray(
            dq.reshape(OH, P, NO, P).transpose(0, 3, 2, 1)).reshape(OH, P, NO * P)
        gs = np.ascontiguousarray(gate_scale[lo:hi].reshape(NO, P).T)
        us = np.ascontiguousarray(up_scale[lo:hi].reshape(NO, P).T)
        packs.append(dict(gw=gw, uw=uw, dw=dw, gs=gs, us=us))

    in_maps = []
    for g in range(dp):
        xg = x_flat[g * m:(g + 1) * m]  # [m, hidden]
        xT = np.ascontiguousarray(xg.T.astype(BF16)).reshape(P * KH, m)
        # [hidden, m] with hidden = k*P + p -> [P, KH, m]
        xT = np.ascontiguousarray(
            xT.reshape(KH, P, m).transpose(1, 0, 2))
        for s in range(tp):
            in_maps.append({"xT": xT, **packs[s]})
    return in_maps


_NC_CACHE = {}


def _get_module():
    key = "full"
    if key not in _NC_CACHE:
        _NC_CACHE[key] = build_module(HIDDEN, INTER // TP, (B * S) // DP)
    return _NC_CACHE[key]


def kernel(x, gate_wq, gate_scale, up_wq, up_scale, down_wq, down_scale,
           _return_results=False):
    x = np.asarray(x)
    x_flat = x.reshape(B * S, HIDDEN)
    in_maps = prep_core_inputs(
        x_flat, np.asarray(gate_wq), np.asarray(gate_scale),
        np.asarray(up_wq), np.asarray(up_scale), np.asarray(down_wq),
        HIDDEN, INTER, DP, TP)

    nc = _get_module()
    res = run_bass_kernel_spmd(nc, in_maps, list(range(N_CORES)))

    m = (B * S) // DP
    y = np.empty((B * S, HIDDEN), np.float32)
    ds = np.asarray(down_scale).astype(np.float32)
    for g in range(DP):
        acc = None
        for s in range(TP):
            part = res.results[g * TP + s]["y"]  # [OH, P, m]
            acc = part if acc is None else acc + part
        # [OH, P, m] -> [hidden, m] -> [m, hidden]
        y[g * m:(g + 1) * m] = acc.reshape(HIDDEN, m).T
    y *= ds[None, :]
    out = y.reshape(B, S, HIDDEN)
    if _return_results:
        return out, res
    return out


# revision 23
# speedup vs baseline: 1.2716x; 1.0669x over previous
"""Trainium2 Bass kernel for a quantized Mistral-style SwiGLU MLP.

Reference computation (per token x of dim HIDDEN=4096):
    g = x @ (gate_wq * gate_scale[:, None]).T      # [INTER]
    u = x @ (up_wq   * up_scale[:, None]).T        # [INTER]
    h = silu(g) * u
    y = h @ (down_wq * down_scale[:, None]).T      # [HIDDEN]

Sharding across 8 NeuronCores: DP4 (token groups of 2048) x TP2 (intermediate
shards of 7168).  Each core runs the same SPMD program:
  phase 1: xT resident in SBUF as KH per-k chunk tiles (feature-major
           [hid, tok]); gate/up weight tiles stream on the ACT HWDGE queue
           while x streams on the SP queue; PE matmuls accumulate g,u in
           PSUM (k-outer / q-inner: stationary held across the MQ moving
           passes, consecutive matmuls on different PSUM banks); ACT applies
           silu(gate_scale*g); DVE forms h = (up_scale*u) * silu(...) in
           bf16; h goes to a DRAM bounce buffer.  o=0 runs gate/up
           interleaved per k so the PE chases the x chunk DMAs instead of
           waiting for the full x load.
  phase 2: h streamed back per k-quarter as per-kk chunk tiles (sync queue;
           next quarter prefetches during the current one); down weight
           tiles stream on the ACT queue; PE matmuls accumulate each
           quarter's [hid, tok] fp32 partial in PSUM, and the 4 partials are
           summed in DRAM via SWDGE accumulate-DMA (output buffers are
           zero-initialized by the SPMD runner).  The first two output
           tiles of quarter 0 run jointly kk-outer to chase the h chunk
           loads.
Host sums the TP pair, applies down_scale, and re-assembles [B, S, HIDDEN].

All weights are fed to the device as exact bf16 integers (values in
[-128,127] are exactly representable); scales stay fp32 and are applied
per-partition on chip (gate/up) or on host (down).
"""

import numpy as np
import ml_dtypes

import concourse.bacc as bacc
import concourse.mybir as mybir
import concourse.tile as tile
from concourse.bass_utils import run_bass_kernel_spmd

BF16 = ml_dtypes.bfloat16
BF = mybir.dt.bfloat16
F32 = mybir.dt.float32

N_CORES = 8
DP, TP = 4, 2
HIDDEN, INTER = 4096, 14336
B, S = 4, 2048

P = 128
FD = 512  # matmul moving free dim (ISA caps total moving columns at 512)

# walrus ships with the redundant-LDWEIGHTS optimization off; our k-outer /
# q-inner matmul streams repeat each stationary operand MQ times in a row,
# which is exactly the pattern that pass optimizes.  The pass refuses
# modules with standalone Ldweights instructions, but the bass serializer
# unconditionally splits every self-loading Matmult into an Ldweights +
# Matmult pair (each BIR instruction may carry at most one semaphore wait).
# So before invoking walrus we re-merge each pair back into a self-loading
# Matmult, moving excess waits onto a PE EventSemaphore (which may carry
# two), and flip --enable-ldw-opt on.  Any assumption violation falls back
# to the untouched BIR with the flag off.
import json as _json
import os as _os
import concourse.bass_utils as _bass_utils


def _merge_ldweights(bir):
    n_es = 0
    for fn in bir["functions"]:
        for blk in fn["blocks"]:
            insts = blk["instructions"]
            out = []
            pending = None
            for inst in insts:
                if inst["opcode"] == "Ldweights":
                    if pending is not None:
                        return None
                    pending = inst
                    continue
                if pending is not None:
                    if inst["opcode"] != "Matmult":
                        return None
                    if inst.get("ldweights"):
                        return None
                    w_mm = inst["ins"][1]
                    w_ld = pending["ins"][0]
                    if (w_mm["memref"] != w_ld["memref"]
                            or w_mm["offset"] != w_ld["offset"]
                            or w_mm["ap"] != w_ld["ap"]):
                        return None
                    ld_sync = pending.get("sync_info", {})
                    mm_sync = inst.setdefault(
                        "sync_info", {"on_update": [], "on_wait": []})
                    if ld_sync.get("on_update"):
                        return None
                    waits = (ld_sync.get("on_wait", [])
                             + mm_sync.get("on_wait", []))
                    inst["ldweights"] = True
                    if len(waits) <= 1:
                        mm_sync["on_wait"] = waits
                    elif len(waits) <= 2:
                        n_es += 1
                        out.append({
                            "debug": inst.get("debug", 0),
                            "engine": "PE",
                            "ins": [], "outs": [],
                            "name": f"ldwmerge_es_{n_es}",
                            "opcode": "EventSemaphore",
                            "sync_info": {"on_update": [], "on_wait": waits},
                        })
                        mm_sync["on_wait"] = []
                    else:
                        return None
                    pending = None
                out.append(inst)
            if pending is not None:
                return None
            blk["instructions"] = out
    return bir


if not getattr(_bass_utils, "_ldw_opt_patched", False):
    _orig_run_command = _bass_utils.run_command

    def _run_command_ldw(cmd, *args, **kwargs):
        if "--enable-ldw-opt=false" in cmd:
            cwd = kwargs.get("cwd")
            bir_path = _os.path.join(cwd, "bir.json") if cwd else None
            merged = None
            if bir_path and _os.path.exists(bir_path):
                try:
                    with open(bir_path) as f:
                        merged = _merge_ldweights(_json.load(f))
                except Exception:
                    merged = None
            if merged is not None:
                with open(bir_path, "w") as f:
                    _json.dump(merged, f)
                cmd = ["--enable-ldw-opt=true"
                       if c == "--enable-ldw-opt=false" else c for c in cmd]
        return _orig_run_command(cmd, *args, **kwargs)

    _bass_utils.run_command = _run_command_ldw
    _bass_utils._ldw_opt_patched = True


def build_module(hidden, inter_sh, m, kq_splits=None):
    """Build the per-core SPMD Bass module.

    hidden:   full hidden dim (contraction of phase 1, output of phase 2)
    inter_sh: this core's intermediate-dim shard
    m:        tokens per core
    """
    KH = hidden // P        # phase-1 contraction chunks
    NO = inter_sh // P      # phase-1 output tiles (inter)
    OH = hidden // P        # phase-2 output tiles (hid)
    MQ = m // FD            # moving passes per psum row
    KQ = 4 if NO % 4 == 0 else 1   # phase-2 k-quarters (h resident per quarter)
    KK = NO // KQ
    assert m % FD == 0

    nc = bacc.Bacc("TRN2", target_bir_lowering=False, debug=False,
                   num_devices=N_CORES)

    xT_d = nc.dram_tensor("xT", [P, KH, m], BF, kind="ExternalInput").ap()
    gw_d = nc.dram_tensor("gw", [NO, P, KH * P], BF, kind="ExternalInput").ap()
    uw_d = nc.dram_tensor("uw", [NO, P, KH * P], BF, kind="ExternalInput").ap()
    dw_d = nc.dram_tensor("dw", [OH, P, NO * P], BF, kind="ExternalInput").ap()
    gs_d = nc.dram_tensor("gs", [P, NO], F32, kind="ExternalInput").ap()
    us_d = nc.dram_tensor("us", [P, NO], F32, kind="ExternalInput").ap()
    y_d = nc.dram_tensor("y", [OH, P, m], F32, kind="ExternalOutput").ap()

    mult = mybir.AluOpType.mult
    silu = mybir.ActivationFunctionType.Silu

    with tile.TileContext(nc) as tc:
        with tc.tile_pool(name="const", bufs=1) as cpool, \
             tc.tile_pool(name="dram", bufs=1, space="DRAM") as dpool:
            gs_sb = cpool.tile([P, NO], F32, tag="gs")
            us_sb = cpool.tile([P, NO], F32, tag="us")
            nc.sync.dma_start(out=gs_sb[:], in_=gs_d[:])
            nc.sync.dma_start(out=us_sb[:], in_=us_d[:])
            h_d = dpool.tile([NO, P, m], BF)

            # ---------------- phase 1: h = silu(gs*g) * (us*u) ----------
            with tc.tile_pool(name="xp", bufs=1) as xp, \
                 tc.tile_pool(name="wp", bufs=2) as wp, \
                 tc.tile_pool(name="sp", bufs=2) as sp, \
                 tc.tile_pool(name="pp", bufs=1, space="PSUM") as pp:
                xk = []
                for k in range(KH):
                    t = xp.tile([P, m], BF, tag=f"x{k}")
                    nc.sync.dma_start(out=t[:], in_=xT_d[:, k])
                    xk.append(t)
                for o in range(NO):
                    gwt = wp.tile([P, KH * P], BF, tag="gw")
                    uwt = wp.tile([P, KH * P], BF, tag="uw")
                    nc.scalar.dma_start(out=gwt[:], in_=gw_d[o])
                    nc.scalar.dma_start(out=uwt[:], in_=uw_d[o])
                    pg = pp.tile([P, m], F32, tag="pg")
                    pu = pp.tile([P, m], F32, tag="pu")
                    if o == 0:
                        # chase the x chunk DMAs: both matrices per k chunk
                        for k in range(KH):
                            for q in range(MQ):
                                nc.tensor.matmul(
                                    pg[:, q * FD:(q + 1) * FD],
                                    gwt[:, k * P:(k + 1) * P],
                                    xk[k][:, q * FD:(q + 1) * FD],
                                    start=(k == 0), stop=(k == KH - 1))
                            for q in range(MQ):
                                nc.tensor.matmul(
                                    pu[:, q * FD:(q + 1) * FD],
                                    uwt[:, k * P:(k + 1) * P],
                                    xk[k][:, q * FD:(q + 1) * FD],
                                    start=(k == 0), stop=(k == KH - 1))
                    else:
                        for k in range(KH):
                            for q in range(MQ):
                                nc.tensor.matmul(
                                    pg[:, q * FD:(q + 1) * FD],
                                    gwt[:, k * P:(k + 1) * P],
                                    xk[k][:, q * FD:(q + 1) * FD],
                                    start=(k == 0), stop=(k == KH - 1))
                        for k in range(KH):
                            for q in range(MQ):
                                nc.tensor.matmul(
                                    pu[:, q * FD:(q + 1) * FD],
                                    uwt[:, k * P:(k + 1) * P],
                                    xk[k][:, q * FD:(q + 1) * FD],
                                    start=(k == 0), stop=(k == KH - 1))
                    sg = sp.tile([P, m], BF, tag="sg")
                    nc.scalar.activation(sg[:], pg[:], silu,
                                         scale=gs_sb[:, o:o + 1])
                    hb = sp.tile([P, m], BF, tag="hb")
                    nc.vector.scalar_tensor_tensor(
                        hb[:], pu[:], us_sb[:, o:o + 1], sg[:], mult, mult)
                    nc.sync.dma_start(out=h_d[o], in_=hb[:])

            # ---- phase 2: y += h[kq] @ down[kq], DMA-accumulated over kq ----
            with tc.tile_pool(name="hqp", bufs=2) as hqp, \
                 tc.tile_pool(name="dwp", bufs=3) as dwp, \
                 tc.tile_pool(name="yop", bufs=2) as yop, \
                 tc.tile_pool(name="pp2", bufs=2, space="PSUM") as pp2:
                for kq in range(KQ):
                    hk = []
                    for kk in range(KK):
                        t = hqp.tile([P, m], BF, tag=f"h{kk}")
                        nc.sync.dma_start(out=t[:], in_=h_d[kq * KK + kk])
                        hk.append(t)

                    def dw_load(o):
                        dwt = dwp.tile([P, KK * P], BF, tag="dw")
                        nc.scalar.dma_start(
                            out=dwt[:],
                            in_=dw_d[o][:, kq * KK * P:(kq + 1) * KK * P])
                        return dwt

                    def py_out(o, py):
                        yo = yop.tile([P, m], F32, tag="yo")
                        nc.vector.tensor_copy(yo[:], py[:])
                        if KQ == 1:
                            nc.sync.dma_start(out=y_d[o], in_=yo[:])
                        else:
                            # accumulate partials straight into DRAM (SWDGE);
                            # output buffers are zero-init'd by the runner
                            nc.gpsimd.dma_start(out=y_d[o], in_=yo[:],
                                                accum_op=mybir.AluOpType.add)

                    o0 = 0
                    if kq == 0 and OH >= 2 and KQ > 1:
                        # chase the h chunk DMAs: first two output tiles
                        # jointly, kk-outer (8 matmuls per arriving chunk)
                        dwt0, dwt1 = dw_load(0), dw_load(1)
                        py0 = pp2.tile([P, m], F32, tag="py")
                        py1 = pp2.tile([P, m], F32, tag="py")
                        for kk in range(KK):
                            for q in range(MQ):
                                nc.tensor.matmul(
                                    py0[:, q * FD:(q + 1) * FD],
                                    dwt0[:, kk * P:(kk + 1) * P],
                                    hk[kk][:, q * FD:(q + 1) * FD],
                                    start=(kk == 0), stop=(kk == KK - 1))
                            for q in range(MQ):
                                nc.tensor.matmul(
                                    py1[:, q * FD:(q + 1) * FD],
                                    dwt1[:, kk * P:(kk + 1) * P],
                                    hk[kk][:, q * FD:(q + 1) * FD],
                                    start=(kk == 0), stop=(kk == KK - 1))
                        py_out(0, py0)
                        py_out(1, py1)
                        o0 = 2
                    for o in range(o0, OH):
                        dwt = dw_load(o)
                        py = pp2.tile([P, m], F32, tag="py")
                        for kk in range(KK):
                            for q in range(MQ):
                                nc.tensor.matmul(
                                    py[:, q * FD:(q + 1) * FD],
                                    dwt[:, kk * P:(kk + 1) * P],
                                    hk[kk][:, q * FD:(q + 1) * FD],
                                    start=(kk == 0), stop=(kk == KK - 1))
                        py_out(o, py)

    nc.compile()
    return nc


def prep_core_inputs(x_flat, gate_wq, gate_scale, up_wq, up_scale, down_wq,
                     hidden, inter, dp, tp, kq_splits=None):
    """Shard + repack full inputs into per-core input maps (list of dicts)."""
    n_tok = x_flat.shape[0]
    m = n_tok // dp
    inter_sh = inter // tp
    KH = hidden // P
    NO = inter_sh // P
    OH = hidden // P

    # per-TP-shard weight packs (shared by all DP groups)
    packs = []
    for s in range(tp):
        lo, hi = s * inter_sh, (s + 1) * inter_sh
        gq = gate_wq[lo:hi].astype(BF16)
        uq = up_wq[lo:hi].astype(BF16)
        # [o,c,k,p] -> [o,p,k,c] -> [NO, P, KH*P]
        gw = np.ascontiguousarray(
            gq.reshape(NO, P, KH, P).transpose(0, 3, 2, 1)).reshape(NO, P, KH * P)
        uw = np.ascontiguousarray(
            uq.reshape(NO, P, KH, P).transpose(0, 3, 2, 1)).reshape(NO, P, KH * P)
        dq = down_wq[:, lo:hi].astype(BF16)
        # [o,c,j,p] -> [o,p,j,c] -> [OH, P, NO*P]
        dw = np.ascontiguousarray(
            dq.reshape(OH, P, NO, P).transpose(0, 3, 2, 1)).reshape(OH, P, NO * P)
        gs = np.ascontiguousarray(gate_scale[lo:hi].reshape(NO, P).T)
        us = np.ascontiguousarray(up_scale[lo:hi].reshape(NO, P).T)
        packs.append(dict(gw=gw, uw=uw, dw=dw, gs=gs, us=us))

    in_maps = []
    for g in range(dp):
        xg = x_flat[g * m:(g + 1) * m]  # [m, hidden]
        xT = np.ascontiguousarray(xg.T.astype(BF16)).reshape(P * KH, m)
        # [hidden, m] with hidden = k*P + p -> [P, KH, m]
        xT = np.ascontiguousarray(
            xT.reshape(KH, P, m).transpose(1, 0, 2))
        for s in range(tp):
            in_maps.append({"xT": xT, **packs[s]})
    return in_maps


_NC_CACHE = {}


def _get_module():
    key = "full"
    if key not in _NC_CACHE:
        _NC_CACHE[key] = build_module(HIDDEN, INTER // TP, (B * S) // DP)
    return _NC_CACHE[key]


def kernel(x, gate_wq, gate_scale, up_wq, up_scale, down_wq, down_scale,
           _return_results=False):
    x = np.asarray(x)
    x_flat = x.reshape(B * S, HIDDEN)
    in_maps = prep_core_inputs(
        x_flat, np.asarray(gate_wq), np.asarray(gate_scale),
        np.asarray(up_wq), np.asarray(up_scale), np.asarray(down_wq),
        HIDDEN, INTER, DP, TP)

    nc = _get_module()
    res = run_bass_kernel_spmd(nc, in_maps, list(range(N_CORES)))

    m = (B * S) // DP
    y = np.empty((B * S, HIDDEN), np.float32)
    ds = np.asarray(down_scale).astype(np.float32)
    for g in range(DP):
        acc = None
        for s in range(TP):
            part = res.results[g * TP + s]["y"]  # [OH, P, m]
            acc = part if acc is None else acc + part
        # [OH, P, m] -> [hidden, m] -> [m, hidden]
        y[g * m:(g + 1) * m] = acc.reshape(HIDDEN, m).T
    y *= ds[None, :]
    out = y.reshape(B, S, HIDDEN)
    if _return_results:
        return out, res
    return out
